# revision 3
# baseline (speedup 1.0000x reference)
"""Cross-attention layer (vision<->text) on 8 Trainium2 NeuronCores.

Problem: B=16, Sv=St=1024, D=1024, fp32.
  q = vision @ Wq.T + bq            [B,Sv,D]
  k = text   @ Wk.T + bk            [B,St,D]
  v = text   @ Wv.T + bv            [B,St,D]
  scores = q @ k.T / sqrt(D)        [B,Sv,St]
  attn = softmax(scores, -1)
  cross_vision = attn @ v           [B,Sv,D]
  cross_text   = attn.T @ vision    [B,St,D]

Sharding: pure data-parallel over batch, 2 items per core, no collectives.

Key algebraic restructuring vs the straightforward 6-matmul form:
  scores*sqrt(D) = (Xv Wq^T + 1 bq^T)(Xt Wk^T + 1 bk^T)^T
                 = Xv M' Xt^T + (row-const terms) + 1 (bq^T Wk Xt^T)
  with M' = Wq^T Wk.  Row-constant terms cancel in the row softmax, and
  bq = 0 in this problem (host falls back to an exact numpy path if not),
  so on device scores ~ Xv M Xt^T with M = Wq^T Wk / sqrt(D) precomputed
  on the host.  That replaces {Q proj, K proj, scores} (3x 1024^3 matmuls
  per item) with {T1 = Xv M, scores = T1 Xt^T} (2x), a 1/6 FLOP cut.
  bv is added on the host after gather (attn rows sum to 1, exact).

Per-core kernel design (per batch item, PE work in parentheses):
  A. prepT: PE-transpose text -> actT[d, t]          (12.3k cyc)
  B. projV: V[t,dv] = actT-stat @ wvt-moving          (65.5k) -- V lands
     directly in the [t, dv] layout cross_vision needs; no transposes.
  C. prepV: PE-transpose vision -> actV[d, s]         (12.3k)
  D. T1T[d',s] = M-stat @ actV-moving                 (65.5k) -- M streamed
     from DRAM in column blocks, wvt stays SBUF-resident.
  F/G. per s-tile, software-pipelined at depth 2 so the in-order PE
     never waits on ACT exp or on the DVE/ACT ET-evacuation copies:
     scores[s,t] = T1T-stat @ actT-moving             (65.5k total)
     E = exp(scores) on ACT with accum_out row sums; rinv = 1/rowsum
     PE-transpose E row-block -> ET                   (12.3k total)
     CV[s,dv] = ET-stat @ V-moving, rinv at evac      (65.5k total)
     E *= rinv in place (making attn rows, for CT)
  H. CT[t,d] = E'-stat @ vis-moving (vision streamed  (65.5k)
     back in), accumulated over s in 8 PSUM groups.
  All matmuls float32r, moving dim 512 (full 1.0 cyc/row rate).
  Total ~364.5k PE cycles/item = ~304us/core at 2.4 GHz for 2 items.

Emission schedule keeps the PE stream gapless across items: prepT(b+1)
runs between FG(b) and H(b) (its text loads prefetch on the idle load
queues during FG(b)), and H(b)'s first vision tiles are prefetched
before prepT(b+1).  Loads live on the sync/scalar queues, stores on
gpsimd; the final H stores split across two queues to shorten the tail.
"""

import sys

import numpy as np

if "/opt/trn_rl_repo" not in sys.path:
    sys.path.insert(0, "/opt/trn_rl_repo")

import concourse.bass as bass
import concourse.tile as tile
from concourse import bacc
from concourse import mybir

PHASE_MARKS = []  # (phase_name, first_unused_instruction_id) at each boundary

P = 128
B, SEQ, DIM = 16, 1024, 1024
N_CORES = 8
BPC = B // N_CORES  # batch items per core
NT = DIM // P  # 8 tiles of 128 along d
F32 = mybir.dt.float32
F32R = mybir.dt.float32r
AF = mybir.ActivationFunctionType
H = 512  # half of a seq dim / PSUM-bank-sized chunk


class Phases:
    def __init__(self, tc, ident, vis, txt, m_d, wvt_sb, cv_d, ct_d, pools):
        self.tc = tc
        self.nc = tc.nc
        self.ident = ident
        self.vis, self.txt, self.m_d, self.wvt_sb = vis, txt, m_d, wvt_sb
        self.cv_d, self.ct_d = cv_d, ct_d
        (self.p_actT, self.p_ave, self.p_t1, self.p_v, self.p_etb, self.p_mc,
         self.p_in, self.p_vt, self.p_cvs, self.p_cts, self.p_rp, self.p_rv,
         self.pp_t, self.pp_mm) = pools
        self.actT = {}
        self.v_sb = {}
        self.t1 = {}
        self.e_sb = {}
        self.rinv = {}
        self.vt_pre_tiles = {}

    def mark(self, name):
        nid = self.nc._state.next_id()
        PHASE_MARKS.append((name, nid))

    def _prep(self, src_d, pool, tag, b):
        """Transpose the full [SEQ, DIM] tensor into actX[d_in, d_out, seq]."""
        nc = self.nc
        actX = pool.tile([P, NT, SEQ], F32R, name="actX", tag=tag)
        for l in range(NT):
            for hh in range(2):  # two [128, 512] half-row loads, dual queue
                tin = self.p_in.tile([P, H], F32R, name="tin", tag="xin")
                eng = nc.sync if hh == 0 else nc.scalar
                eng.dma_start(
                    out=tin,
                    in_=src_d[b, l * P:(l + 1) * P, hh * H:(hh + 1) * H].bitcast(F32R))
                tp4 = self.pp_t.tile([P, 4, P], F32R, name="tp4", tag="tp4")
                for j in range(4):
                    nc.tensor.matmul(
                        tp4[:, j, :], tin[:, j * P:(j + 1) * P], self.ident,
                        is_transpose=True, start=(j == 0), stop=(j == 3),
                        skip_group_check=True,
                    )
                if hh == 0:
                    nc.vector.tensor_copy(actX[:, 0:4, l * P:(l + 1) * P], tp4)
                else:
                    nc.scalar.copy(actX[:, 4:8, l * P:(l + 1) * P], tp4)
        return actX

    def prepT(self, b):
        self.mark(f"b{b}_prepT")
        self.actT[b] = self._prep(self.txt, self.p_actT, "actT", b)

    def projV(self, b):
        """V[t, dv] = Xt @ Wv^T, direct [t, dv] layout.

        stat = actT t-block (Xt rows), moving = resident wvt columns."""
        self.mark(f"b{b}_projV")
        nc = self.nc
        actT = self.actT[b]
        v_sb = self.p_v.tile([P, NT, SEQ], F32R, name="v_sb", tag="v")
        for tb in range(NT):
            pss = [self.pp_mm.tile([P, H], F32, name=f"ps_v{i}", tag="mm")
                   for i in range(2)]
            for do in range(NT):
                for hh in range(2):
                    nc.tensor.matmul(pss[hh], actT[:, do, tb * P:(tb + 1) * P],
                                     self.wvt_sb[:, do, hh * H:(hh + 1) * H],
                                     start=(do == 0), stop=(do == NT - 1))
            nc.vector.tensor_copy(v_sb[:, tb, 0:H], pss[0])
            nc.scalar.copy(v_sb[:, tb, H:2 * H], pss[1])
        self.v_sb[b] = v_sb

    def prepV(self, b):
        self.mark(f"b{b}_prepV")
        self.actV = self._prep(self.vis, self.p_ave, "ave", b)

    def t1t(self, b):
        """T1T[d', s] = (Xv M)^T = M-colblock-stat @ actV, M streamed."""
        self.mark(f"b{b}_T1")
        nc = self.nc
        t1 = self.p_t1.tile([P, NT, SEQ], F32R, name="t1", tag="t1")
        for eo in range(NT):
            mc = self.p_mc.tile([P, NT, P], F32R, name="mc", tag="mc")
            nc.scalar.dma_start(
                out=mc,
                in_=self.m_d[:, eo * P:(eo + 1) * P]
                    .rearrange("(do di) e -> di do e", di=P),
            )
            pss = [self.pp_mm.tile([P, H], F32, name=f"ps_t{i}", tag="mm")
                   for i in range(2)]
            for do in range(NT):
                for hh in range(2):
                    nc.tensor.matmul(pss[hh], mc[:, do, :],
                                     self.actV[:, do, hh * H:(hh + 1) * H],
                                     start=(do == 0), stop=(do == NT - 1))
            nc.vector.tensor_copy(t1[:, eo, 0:H], pss[0])
            nc.scalar.copy(t1[:, eo, H:2 * H], pss[1])
        self.t1[b] = t1

    def fg(self, b):
        """scores -> exp/rowsum -> ET -> cross_vision, depth-2 pipelined.

        PE order per steady-state iteration: scores(so+2) matmuls,
        E-transposes(so+1), CV(so) matmuls.  exp(so+2) runs on ACT during
        the next iteration's scores; the ET copies of so+1 complete during
        scores(so+2)/CV(so), so CV(so+1) never waits on them."""
        self.mark(f"b{b}_F")
        nc = self.nc
        t1, actT, v_sb = self.t1[b], self.actT[b], self.v_sb[b]
        e_sb = self.p_ave.tile([P, NT, SEQ], F32R, name="e_sb", tag="ave")
        rinv = self.p_rv.tile([P, NT], F32, name="rinv", tag="rinv")
        self.e_sb[b] = e_sb
        rps = {}
        etbs = {}

        def scores_stile(so):
            rp = self.p_rp.tile([P, 2], F32, name="rp", tag="rp")
            pss = [self.pp_mm.tile([P, H], F32, name=f"ps_s{i}", tag="mm")
                   for i in range(2)]
            for do in range(NT):
                for tc_ in range(2):
                    nc.tensor.matmul(pss[tc_], t1[:, do, so * P:(so + 1) * P],
                                     actT[:, do, tc_ * H:(tc_ + 1) * H],
                                     start=(do == 0), stop=(do == NT - 1))
            for tc_ in range(2):
                nc.scalar.activation(out=e_sb[:, so, tc_ * H:(tc_ + 1) * H],
                                     in_=pss[tc_], func=AF.Exp,
                                     accum_out=rp[:, tc_:tc_ + 1])
            rps[so] = rp

        def etp(so):
            rp = rps.pop(so)
            rsum = self.p_rp.tile([P, 1], F32, name="rsum", tag="rsum")
            nc.vector.tensor_add(rsum, rp[:, 0:1], rp[:, 1:2])
            nc.vector.reciprocal(rinv[:, so:so + 1], rsum)
            # transpose the *unnormalized* E row-block; copies split across
            # DVE and ACT so CV's first stat is ready fast
            etb = self.p_etb.tile([P, NT, P], F32R, name="etb", tag="etb")
            for tg in range(2):
                tp4 = self.pp_t.tile([P, 4, P], F32R, name="tp4e", tag="tp4")
                for j in range(4):
                    tt = tg * 4 + j
                    nc.tensor.matmul(tp4[:, j, :], e_sb[:, so, tt * P:(tt + 1) * P],
                                     self.ident, is_transpose=True,
                                     start=(j == 0), stop=(j == 3),
                                     skip_group_check=True)
                nc.vector.tensor_copy(etb[:, tg * 4:tg * 4 + 2, :], tp4[:, 0:2, :])
                nc.scalar.copy(etb[:, tg * 4 + 2:tg * 4 + 4, :], tp4[:, 2:4, :])
            etbs[so] = etb

        def cv(so):
            etb = etbs.pop(so)
            pcv = [self.pp_mm.tile([P, H], F32, name=f"ps_cv{i}", tag="mm")
                   for i in range(2)]
            for tt in range(NT):
                for dc in range(2):
                    nc.tensor.matmul(pcv[dc], etb[:, tt, :],
                                     v_sb[:, tt, dc * H:(dc + 1) * H],
                                     start=(tt == 0), stop=(tt == NT - 1))
            for dc in range(2):
                cvs = self.p_cvs.tile([P, H], F32, name="cvs", tag="cvs")
                nc.scalar.mul(cvs, pcv[dc], mul=rinv[:, so:so + 1])
                nc.gpsimd.dma_start(
                    out=self.cv_d[b, so * P:(so + 1) * P, dc * H:(dc + 1) * H],
                    in_=cvs)
            # normalize this E row-block in place (for cross_text later)
            nc.vector.tensor_scalar_mul(e_sb[:, so, :], e_sb[:, so, :],
                                        scalar1=rinv[:, so:so + 1])

        scores_stile(0)
        scores_stile(1)
        etp(0)
        for so in range(NT):
            if so + 2 < NT:
                scores_stile(so + 2)
            if so + 1 < NT:
                etp(so + 1)
            cv(so)
        self.rinv[b] = rinv

    def vt_pre(self, b):
        """Prefetch H(b)'s first two vision tiles on the idle load queues."""
        nc = self.nc
        pre = []
        for so in range(2):
            vt = self.p_vt.tile([P, H], F32R, name="vt", tag="vt")
            eng = nc.sync if so % 2 == 0 else nc.scalar
            eng.dma_start(out=vt, in_=self.vis[b, so * P:(so + 1) * P, 0:H]
                          .bitcast(F32R))
            pre.append(vt)
        self.vt_pre_tiles[b] = pre

    def h(self, b, last):
        """cross_text = E'.T @ Xv (E' already rinv-scaled).

        8 concurrent PSUM accumulation groups (6 from pmm + 2 borrowed from
        the transpose pool): each vision tile load feeds 8 matmuls, vision
        read once per d-half.  dc=0 stores go to the gpsimd queue only (so
        dc=1's loads aren't queued behind them); the final dc=1 stores split
        across sync+gpsimd to shorten the kernel tail."""
        self.mark(f"b{b}_H")
        nc = self.nc
        e_sb = self.e_sb[b]
        pre = self.vt_pre_tiles.pop(b, [])
        for dc in range(2):
            pss = [self.pp_mm.tile([P, H], F32, name=f"ps_ct{i}", tag="mm")
                   for i in range(6)]
            pss += [self.pp_t.tile([P, H], F32, name=f"ps_ct{i + 6}", tag="tp4")
                    for i in range(2)]
            for so in range(NT):
                if dc == 0 and so < len(pre):
                    vt = pre[so]
                else:
                    vt = self.p_vt.tile([P, H], F32R, name="vt", tag="vt")
                    eng = nc.sync if so % 2 == 0 else nc.scalar
                    eng.dma_start(
                        out=vt,
                        in_=self.vis[b, so * P:(so + 1) * P, dc * H:(dc + 1) * H]
                            .bitcast(F32R))
                for tt in range(NT):
                    nc.tensor.matmul(pss[tt], e_sb[:, so, tt * P:(tt + 1) * P], vt,
                                     start=(so == 0), stop=(so == NT - 1))
            final = last and dc == 1
            for tt in range(NT):
                cts = self.p_cts.tile([P, H], F32, name="cts", tag="cts")
                if tt % 2 == 0:
                    nc.vector.tensor_copy(cts, pss[tt])
                else:
                    nc.scalar.copy(cts, pss[tt])
                eng = nc.sync if (final and tt % 2 == 0) else nc.gpsimd
                eng.dma_start(
                    out=self.ct_d[b, tt * P:(tt + 1) * P, dc * H:(dc + 1) * H],
                    in_=cts)
        self.mark(f"b{b}_end")


def build_nc():
    nc = bacc.Bacc("TRN2", target_bir_lowering=False, debug=False, num_devices=N_CORES)
    vis = nc.dram_tensor("vision", [BPC, SEQ, DIM], F32, kind="ExternalInput").ap()
    txt = nc.dram_tensor("text", [BPC, SEQ, DIM], F32, kind="ExternalInput").ap()
    m_d = nc.dram_tensor("m_mat", [DIM, DIM], F32R, kind="ExternalInput").ap()
    wvt_d = nc.dram_tensor("wvt", [DIM, DIM], F32R, kind="ExternalInput").ap()
    id_d = nc.dram_tensor("ident128", [P, P], F32R, kind="ExternalInput").ap()
    cv_d = nc.dram_tensor("cross_vision", [BPC, SEQ, DIM], F32, kind="ExternalOutput").ap()
    ct_d = nc.dram_tensor("cross_text", [BPC, SEQ, DIM], F32, kind="ExternalOutput").ap()

    with tile.TileContext(nc) as tc:
        import contextlib
        with contextlib.ExitStack() as ctx:
            def sp(name, bufs):
                return ctx.enter_context(tc.tile_pool(name=name, bufs=bufs))

            p_actT = sp("actT", 1)   # 32KB/part: Xt^T
            p_ave = sp("ave", 1)     # 32KB/part: actV then e_sb (disjoint lives)
            p_t1 = sp("t1", 1)       # 32KB/part: T1^T
            p_v = sp("v", 1)         # 32KB/part: V[t, dv]
            p_w = sp("w", 1)         # 32KB/part: resident Wv^T
            p_etb = sp("etb", 2)     # 8KB/part: ET blocks, depth-2 pipeline
            p_mc = sp("mc", 2)       # 8KB/part: M column-block staging
            p_in = sp("xin", 6)      # 12KB/part: prep load staging
            p_vt = sp("vt", 4)       # 8KB/part: vision re-stream staging
            p_cvs = sp("cvs", 2)     # 4KB/part
            p_cts = sp("cts", 2)     # 4KB/part
            p_rp = sp("rp", 4)
            p_rv = sp("rv", 2)
            p_sm = sp("sm", 1)
            pp_t = ctx.enter_context(
                tc.tile_pool(name="pp_t", bufs=2, space=bass.MemorySpace.PSUM))
            pp_mm = ctx.enter_context(
                tc.tile_pool(name="pp_mm", bufs=6, space=bass.MemorySpace.PSUM))

            ident = p_sm.tile([P, P], F32R, name="ident")
            nc.sync.dma_start(out=ident, in_=id_d)
            # resident Wv^T [di, do, e]: 8 plain contiguous 512KB loads
            wvt_sb = p_w.tile([P, NT, SEQ], F32R, name="wvt_sb", tag="wvt")
            for do in range(NT):
                nc.gpsimd.dma_start(out=wvt_sb[:, do, :],
                                    in_=wvt_d[do * P:(do + 1) * P, :])

            pools = (p_actT, p_ave, p_t1, p_v, p_etb, p_mc, p_in, p_vt,
                     p_cvs, p_cts, p_rp, p_rv, pp_t, pp_mm)
            ph = Phases(tc, ident, vis, txt, m_d, wvt_sb, cv_d, ct_d, pools)
            ph.prepT(0)
            for b in range(BPC):
                ph.projV(b)
                ph.prepV(b)
                ph.t1t(b)
                ph.fg(b)
                ph.vt_pre(b)
                if b + 1 < BPC:
                    ph.prepT(b + 1)
                ph.h(b, last=(b + 1 == BPC))
    nc.compile()
    return nc


_NC_CACHE = None


def _get_nc():
    global _NC_CACHE
    if _NC_CACHE is None:
        _NC_CACHE = build_nc()
    return _NC_CACHE


def _reference_numpy(vision_repr, text_repr, Wq, bq, Wk, bk, Wv, bv):
    """Exact fallback (never hit for this problem's inputs: bq == 0)."""
    Xv = np.asarray(vision_repr, np.float64)
    Xt = np.asarray(text_repr, np.float64)
    q = Xv @ np.asarray(Wq, np.float64).T + np.asarray(bq, np.float64)
    k = Xt @ np.asarray(Wk, np.float64).T + np.asarray(bk, np.float64)
    v = Xt @ np.asarray(Wv, np.float64).T + np.asarray(bv, np.float64)
    s = np.einsum("bsd,btd->bst", q, k) / np.sqrt(np.float64(Xv.shape[-1]))
    s -= s.max(axis=-1, keepdims=True)
    e = np.exp(s)
    attn = e / e.sum(axis=-1, keepdims=True)
    cv = np.einsum("bst,btd->bsd", attn, v)
    ct = np.einsum("bst,bsd->btd", attn, Xv)
    return cv.astype(np.float32), ct.astype(np.float32)


def make_in_maps(vision_repr, text_repr, Wq, bq, Wk, bk, Wv, bv):
    s = 1.0 / np.sqrt(np.float64(DIM))
    m = np.ascontiguousarray(
        (np.asarray(Wq, np.float64).T @ np.asarray(Wk, np.float64) * s)
        .astype(np.float32))
    wvt = np.ascontiguousarray(np.asarray(Wv, np.float32).T)
    vis = np.asarray(vision_repr, np.float32)
    txt = np.asarray(text_repr, np.float32)
    in_maps = []
    for c in range(N_CORES):
        in_maps.append({
            "vision": vis[c * BPC:(c + 1) * BPC],
            "text": txt[c * BPC:(c + 1) * BPC],
            "m_mat": m, "wvt": wvt,
            "ident128": np.eye(P, dtype=np.float32),
        })
    return in_maps


def kernel(vision_repr, text_repr, Wq, bq, Wk, bk, Wv, bv):
    if np.max(np.abs(np.asarray(bq, np.float32))) != 0.0:
        # bq feeds a softmax-variant term the fused-M device path drops;
        # exact host fallback (not hit for this problem: bq is zeros).
        return _reference_numpy(vision_repr, text_repr, Wq, bq, Wk, bk, Wv, bv)

    from concourse.bass_utils import run_bass_kernel_spmd

    nc = _get_nc()
    in_maps = make_in_maps(vision_repr, text_repr, Wq, bq, Wk, bk, Wv, bv)
    res = run_bass_kernel_spmd(nc, in_maps, list(range(N_CORES))).results
    cv = np.concatenate([r_["cross_vision"] for r_ in res], axis=0)
    ct = np.concatenate([r_["cross_text"] for r_ in res], axis=0)
    cv = cv + np.asarray(bv, np.float32)[None, None, :]
    return cv, ct


# revision 6
# speedup vs baseline: 1.0677x; 1.0677x over previous
"""Cross-attention layer (vision<->text) on 8 Trainium2 NeuronCores.

Problem: B=16, Sv=St=1024, D=1024, fp32.
  q = vision @ Wq.T + bq            [B,Sv,D]
  k = text   @ Wk.T + bk            [B,St,D]
  v = text   @ Wv.T + bv            [B,St,D]
  scores = q @ k.T / sqrt(D)        [B,Sv,St]
  attn = softmax(scores, -1)
  cross_vision = attn @ v           [B,Sv,D]
  cross_text   = attn.T @ vision    [B,St,D]

Sharding: pure data-parallel over batch, 2 items per core, no collectives.

Key algebraic restructuring vs the straightforward 6-matmul form:
  scores*sqrt(D) = (Xv Wq^T + 1 bq^T)(Xt Wk^T + 1 bk^T)^T
                 = Xv M' Xt^T + (row-const terms) + 1 (bq^T Wk Xt^T)
  with M' = Wq^T Wk.  Row-constant terms cancel in the row softmax, and
  bq = 0 in this problem (host falls back to an exact numpy path if not),
  so on device scores ~ Xv M Xt^T with M = Wq^T Wk / sqrt(D) precomputed
  on the host.  That replaces {Q proj, K proj, scores} (3x 1024^3 matmuls
  per item) with {T1 = Xv M, scores = T1 Xt^T} (2x), a 1/6 FLOP cut.
  bv is added on the host after gather (attn rows sum to 1, exact).

Per-core kernel design (per batch item, PE work in parentheses):
  A. prepT: PE-transpose text -> actT[d, t]          (12.3k cyc)
  B. projV: V[t,dv] = actT-stat @ wvt-moving          (65.5k) -- V lands
     directly in the [t, dv] layout cross_vision needs; no transposes.
  C. prepV: PE-transpose vision -> actV[d, s]         (12.3k)
  D. T1T[d',s] = M-stat @ actV-moving                 (65.5k) -- M streamed
     from DRAM in column blocks, wvt stays SBUF-resident.
  F/G. per s-tile, software-pipelined at depth 2 so the in-order PE
     never waits on ACT exp or on the DVE/ACT ET-evacuation copies:
     scores[s,t] = T1T-stat @ actT-moving             (65.5k total)
     E = exp(scores) on ACT with accum_out row sums; rinv = 1/rowsum
     PE-transpose E row-block -> ET                   (12.3k total)
     CV[s,dv] = ET-stat @ V-moving, rinv at evac      (65.5k total)
     E *= rinv in place (making attn rows, for CT)
  H. CT[t,d] = E'-stat @ vis-moving (vision streamed  (65.5k)
     back in), accumulated over s in 8 PSUM groups.
  All matmuls float32r, moving dim 512 (full 1.0 cyc/row rate).
  Total ~364.5k PE cycles/item = ~304us/core at 2.4 GHz for 2 items.

Emission schedule keeps the PE stream gapless across items: prepT(b+1)
runs between FG(b) and H(b) (its text loads prefetch on the idle load
queues during FG(b)), and H(b)'s first vision tiles are prefetched
before prepT(b+1).  Loads live on the sync/scalar queues, stores on
gpsimd; the final H stores split across two queues to shorten the tail.
"""

import sys

import numpy as np

if "/opt/trn_rl_repo" not in sys.path:
    sys.path.insert(0, "/opt/trn_rl_repo")

import concourse.bass as bass
import concourse.tile as tile
from concourse import bacc
from concourse import mybir

PHASE_MARKS = []  # (phase_name, first_unused_instruction_id) at each boundary

P = 128
B, SEQ, DIM = 16, 1024, 1024
N_CORES = 8
BPC = B // N_CORES  # batch items per core
NT = DIM // P  # 8 tiles of 128 along d
F32 = mybir.dt.float32
F32R = mybir.dt.float32r
AF = mybir.ActivationFunctionType
H = 512  # half of a seq dim / PSUM-bank-sized chunk


class Phases:
    def __init__(self, tc, ident, vis, txt, m_d, wvt_sb, cv_d, ct_d, pools):
        self.tc = tc
        self.nc = tc.nc
        self.ident = ident
        self.vis, self.txt, self.m_d, self.wvt_sb = vis, txt, m_d, wvt_sb
        self.cv_d, self.ct_d = cv_d, ct_d
        (self.p_actT, self.p_ave, self.p_t1, self.p_v, self.p_etb, self.p_mc,
         self.p_in, self.p_vt, self.p_cvs, self.p_cts, self.p_rp, self.p_rv,
         self.pp_t, self.pp_mm) = pools
        self.actT = {}
        self.v_sb = {}
        self.t1 = {}
        self.e_sb = {}
        self.rinv = {}
        self.vt_pre_tiles = {}

    def mark(self, name):
        nid = self.nc._state.next_id()
        PHASE_MARKS.append((name, nid))

    def _prep(self, src_d, pool, tag, b):
        """Transpose the full [SEQ, DIM] tensor into actX[d_in, d_out, seq]."""
        nc = self.nc
        actX = pool.tile([P, NT, SEQ], F32R, name="actX", tag=tag)
        for l in range(NT):
            for hh in range(2):  # two [128, 512] half-row loads, dual queue
                tin = self.p_in.tile([P, H], F32R, name="tin", tag="xin")
                eng = nc.sync if hh == 0 else nc.scalar
                eng.dma_start(
                    out=tin,
                    in_=src_d[b, l * P:(l + 1) * P, hh * H:(hh + 1) * H].bitcast(F32R))
                tp4 = self.pp_t.tile([P, 4, P], F32R, name="tp4", tag="tp4")
                for j in range(4):
                    nc.tensor.matmul(
                        tp4[:, j, :], tin[:, j * P:(j + 1) * P], self.ident,
                        is_transpose=True, start=(j == 0), stop=(j == 3),
                        skip_group_check=True,
                    )
                if hh == 0:
                    nc.vector.tensor_copy(actX[:, 0:4, l * P:(l + 1) * P], tp4)
                else:
                    nc.scalar.copy(actX[:, 4:8, l * P:(l + 1) * P], tp4)
        return actX

    def prepT(self, b):
        self.mark(f"b{b}_prepT")
        self.actT[b] = self._prep(self.txt, self.p_actT, "actT", b)

    def projV(self, b):
        """V[t, dv] = Xt @ Wv^T, direct [t, dv] layout.

        stat = actT t-block (Xt rows), moving = resident wvt columns."""
        self.mark(f"b{b}_projV")
        nc = self.nc
        actT = self.actT[b]
        v_sb = self.p_v.tile([P, NT, SEQ], F32R, name="v_sb", tag="v")
        for tb in range(NT):
            pss = [self.pp_mm.tile([P, H], F32, name=f"ps_v{i}", tag="mm")
                   for i in range(2)]
            for do in range(NT):
                for hh in range(2):
                    nc.tensor.matmul(pss[hh], actT[:, do, tb * P:(tb + 1) * P],
                                     self.wvt_sb[:, do, hh * H:(hh + 1) * H],
                                     start=(do == 0), stop=(do == NT - 1))
            nc.vector.tensor_copy(v_sb[:, tb, 0:H], pss[0])
            nc.scalar.copy(v_sb[:, tb, H:2 * H], pss[1])
        self.v_sb[b] = v_sb

    def prepV(self, b):
        self.mark(f"b{b}_prepV")
        self.actV = self._prep(self.vis, self.p_ave, "ave", b)

    def t1t(self, b):
        """T1T[d', s] = (Xv M)^T = M-colblock-stat @ actV, M streamed."""
        self.mark(f"b{b}_T1")
        nc = self.nc
        t1 = self.p_t1.tile([P, NT, SEQ], F32R, name="t1", tag="t1")
        for eo in range(NT):
            mc = self.p_mc.tile([P, NT, P], F32R, name="mc", tag="mc")
            nc.scalar.dma_start(
                out=mc,
                in_=self.m_d[:, eo * P:(eo + 1) * P]
                    .rearrange("(do di) e -> di do e", di=P),
            )
            pss = [self.pp_mm.tile([P, H], F32, name=f"ps_t{i}", tag="mm")
                   for i in range(2)]
            for do in range(NT):
                for hh in range(2):
                    nc.tensor.matmul(pss[hh], mc[:, do, :],
                                     self.actV[:, do, hh * H:(hh + 1) * H],
                                     start=(do == 0), stop=(do == NT - 1))
            nc.vector.tensor_copy(t1[:, eo, 0:H], pss[0])
            nc.scalar.copy(t1[:, eo, H:2 * H], pss[1])
        self.t1[b] = t1

    def fg(self, b):
        """scores -> exp/rowsum -> ET -> cross_vision, depth-2 pipelined.

        PE order per steady-state iteration: scores(so+2) matmuls,
        E-transposes(so+1), CV(so) matmuls.  exp(so+2) runs on ACT during
        the next iteration's scores; the ET copies of so+1 complete during
        scores(so+2)/CV(so), so CV(so+1) never waits on them."""
        self.mark(f"b{b}_F")
        nc = self.nc
        t1, actT, v_sb = self.t1[b], self.actT[b], self.v_sb[b]
        e_sb = self.p_ave.tile([P, NT, SEQ], F32R, name="e_sb", tag="ave")
        rinv = self.p_rv.tile([P, NT], F32, name="rinv", tag="rinv")
        self.e_sb[b] = e_sb
        rps = {}
        etbs = {}

        def scores_stile(so):
            rp = self.p_rp.tile([P, 2], F32, name="rp", tag="rp")
            pss = [self.pp_mm.tile([P, H], F32, name=f"ps_s{i}", tag="mm")
                   for i in range(2)]
            for do in range(NT):
                for tc_ in range(2):
                    nc.tensor.matmul(pss[tc_], t1[:, do, so * P:(so + 1) * P],
                                     actT[:, do, tc_ * H:(tc_ + 1) * H],
                                     start=(do == 0), stop=(do == NT - 1))
            for tc_ in range(2):
                nc.scalar.activation(out=e_sb[:, so, tc_ * H:(tc_ + 1) * H],
                                     in_=pss[tc_], func=AF.Exp,
                                     accum_out=rp[:, tc_:tc_ + 1])
            rps[so] = rp

        def etp(so):
            rp = rps.pop(so)
            rsum = self.p_rp.tile([P, 1], F32, name="rsum", tag="rsum")
            nc.vector.tensor_add(rsum, rp[:, 0:1], rp[:, 1:2])
            nc.vector.reciprocal(rinv[:, so:so + 1], rsum)
            # transpose the *unnormalized* E row-block; copies split across
            # DVE and ACT so CV's first stat is ready fast
            etb = self.p_etb.tile([P, NT, P], F32R, name="etb", tag="etb")
            for tg in range(2):
                tp4 = self.pp_t.tile([P, 4, P], F32R, name="tp4e", tag="tp4")
                for j in range(4):
                    tt = tg * 4 + j
                    nc.tensor.matmul(tp4[:, j, :], e_sb[:, so, tt * P:(tt + 1) * P],
                                     self.ident, is_transpose=True,
                                     start=(j == 0), stop=(j == 3),
                                     skip_group_check=True)
                nc.vector.tensor_copy(etb[:, tg * 4:tg * 4 + 2, :], tp4[:, 0:2, :])
                nc.scalar.copy(etb[:, tg * 4 + 2:tg * 4 + 4, :], tp4[:, 2:4, :])
            etbs[so] = etb

        def cv(so):
            etb = etbs.pop(so)
            pcv = [self.pp_mm.tile([P, H], F32, name=f"ps_cv{i}", tag="mm")
                   for i in range(2)]
            for tt in range(NT):
                for dc in range(2):
                    nc.tensor.matmul(pcv[dc], etb[:, tt, :],
                                     v_sb[:, tt, dc * H:(dc + 1) * H],
                                     start=(tt == 0), stop=(tt == NT - 1))
            for dc in range(2):
                cvs = self.p_cvs.tile([P, H], F32, name="cvs", tag="cvs")
                nc.scalar.mul(cvs, pcv[dc], mul=rinv[:, so:so + 1])
                nc.gpsimd.dma_start(
                    out=self.cv_d[b, so * P:(so + 1) * P, dc * H:(dc + 1) * H],
                    in_=cvs)
            # normalize this E row-block in place (for cross_text later)
            nc.vector.tensor_scalar_mul(e_sb[:, so, :], e_sb[:, so, :],
                                        scalar1=rinv[:, so:so + 1])

        scores_stile(0)
        scores_stile(1)
        etp(0)
        for so in range(NT):
            if so + 2 < NT:
                scores_stile(so + 2)
            if so + 1 < NT:
                etp(so + 1)
            cv(so)
        self.rinv[b] = rinv

    def _h_pass(self, b, dc, tts, borrow_t, final):
        """One cross_text accumulation pass: groups for t-blocks `tts`,
        accumulated over all of s, moving = streamed vision d-half dc."""
        nc = self.nc
        e_sb = self.e_sb[b]
        pss = []
        for i, tt in enumerate(tts):
            if i < borrow_t:
                # borrowed transpose-pool banks go FIRST so their evac
                # copies free them immediately for the next prep phase
                pss.append(self.pp_t.tile([P, H], F32, name=f"ps_ct{tt}",
                                          tag="tp4"))
            else:
                pss.append(self.pp_mm.tile([P, H], F32, name=f"ps_ct{tt}",
                                           tag="mm"))
        for so in range(NT):
            vt = self.p_vt.tile([P, H], F32R, name="vt", tag="vt")
            eng = nc.sync if so % 2 == 0 else nc.scalar
            eng.dma_start(
                out=vt,
                in_=self.vis[b, so * P:(so + 1) * P, dc * H:(dc + 1) * H]
                    .bitcast(F32R))
            for i, tt in enumerate(tts):
                nc.tensor.matmul(pss[i], e_sb[:, so, tt * P:(tt + 1) * P], vt,
                                 start=(so == 0), stop=(so == NT - 1))
        for i, tt in enumerate(tts):
            cts = self.p_cts.tile([P, H], F32, name="cts", tag="cts")
            if i % 2 == 0:
                nc.vector.tensor_copy(cts, pss[i])
            else:
                nc.scalar.copy(cts, pss[i])
            eng = nc.sync if (final and i % 2 == 0) else nc.gpsimd
            eng.dma_start(
                out=self.ct_d[b, tt * P:(tt + 1) * P, dc * H:(dc + 1) * H],
                in_=cts)

    def h(self, b, last):
        """cross_text = E'.T @ Xv (E' already rinv-scaled).

        8 concurrent PSUM accumulation groups (2 borrowed from the transpose
        pool, evacuated first): each vision tile load feeds 8 matmuls,
        vision read once per d-half.  For the very last d-half of the last
        item the pass is split into two 4-group halves so the final stores
        drain behind live matmuls, shortening the kernel tail; its stores
        also split across the sync+gpsimd queues."""
        self.mark(f"b{b}_H")
        self._h_pass(b, 0, list(range(NT)), borrow_t=2, final=False)
        if not last:
            self._h_pass(b, 1, list(range(NT)), borrow_t=2, final=False)
        else:
            self._h_pass(b, 1, [0, 1, 2, 3], borrow_t=2, final=False)
            self._h_pass(b, 1, [4, 5, 6, 7], borrow_t=0, final=True)
        self.mark(f"b{b}_end")


def build_nc():
    nc = bacc.Bacc("TRN2", target_bir_lowering=False, debug=False, num_devices=N_CORES)
    vis = nc.dram_tensor("vision", [BPC, SEQ, DIM], F32, kind="ExternalInput").ap()
    txt = nc.dram_tensor("text", [BPC, SEQ, DIM], F32, kind="ExternalInput").ap()
    m_d = nc.dram_tensor("m_mat", [DIM, DIM], F32R, kind="ExternalInput").ap()
    wvt_d = nc.dram_tensor("wvt", [DIM, DIM], F32R, kind="ExternalInput").ap()
    id_d = nc.dram_tensor("ident128", [P, P], F32R, kind="ExternalInput").ap()
    cv_d = nc.dram_tensor("cross_vision", [BPC, SEQ, DIM], F32, kind="ExternalOutput").ap()
    ct_d = nc.dram_tensor("cross_text", [BPC, SEQ, DIM], F32, kind="ExternalOutput").ap()

    with tile.TileContext(nc) as tc:
        import contextlib
        with contextlib.ExitStack() as ctx:
            def sp(name, bufs):
                return ctx.enter_context(tc.tile_pool(name=name, bufs=bufs))

            p_actT = sp("actT", 1)   # 32KB/part: Xt^T
            p_ave = sp("ave", 1)     # 32KB/part: actV then e_sb (disjoint lives)
            p_t1 = sp("t1", 1)       # 32KB/part: T1^T
            p_v = sp("v", 1)         # 32KB/part: V[t, dv]
            p_w = sp("w", 1)         # 32KB/part: resident Wv^T
            p_etb = sp("etb", 2)     # 8KB/part: ET blocks, depth-2 pipeline
            p_mc = sp("mc", 2)       # 8KB/part: M column-block staging
            p_in = sp("xin", 5)      # 10KB/part: prep load staging
            p_vt = sp("vt", 4)       # 8KB/part: vision re-stream staging
            p_cvs = sp("cvs", 2)     # 4KB/part
            p_cts = sp("cts", 4)     # 8KB/part
            p_rp = sp("rp", 4)
            p_rv = sp("rv", 2)
            p_sm = sp("sm", 1)
            pp_t = ctx.enter_context(
                tc.tile_pool(name="pp_t", bufs=2, space=bass.MemorySpace.PSUM))
            pp_mm = ctx.enter_context(
                tc.tile_pool(name="pp_mm", bufs=6, space=bass.MemorySpace.PSUM))

            ident = p_sm.tile([P, P], F32R, name="ident")
            nc.sync.dma_start(out=ident, in_=id_d)
            # resident Wv^T [di, do, e]: 8 plain contiguous 512KB loads
            wvt_sb = p_w.tile([P, NT, SEQ], F32R, name="wvt_sb", tag="wvt")
            for do in range(NT):
                nc.gpsimd.dma_start(out=wvt_sb[:, do, :],
                                    in_=wvt_d[do * P:(do + 1) * P, :])

            pools = (p_actT, p_ave, p_t1, p_v, p_etb, p_mc, p_in, p_vt,
                     p_cvs, p_cts, p_rp, p_rv, pp_t, pp_mm)
            ph = Phases(tc, ident, vis, txt, m_d, wvt_sb, cv_d, ct_d, pools)
            ph.prepT(0)
            for b in range(BPC):
                ph.projV(b)
                ph.prepV(b)
                ph.t1t(b)
                ph.fg(b)
                ph.h(b, last=(b + 1 == BPC))
                if b + 1 < BPC:
                    ph.prepT(b + 1)
    nc.compile()
    return nc


_NC_CACHE = None


def _get_nc():
    global _NC_CACHE
    if _NC_CACHE is None:
        _NC_CACHE = build_nc()
    return _NC_CACHE


def _reference_numpy(vision_repr, text_repr, Wq, bq, Wk, bk, Wv, bv):
    """Exact fallback (never hit for this problem's inputs: bq == 0)."""
    Xv = np.asarray(vision_repr, np.float64)
    Xt = np.asarray(text_repr, np.float64)
    q = Xv @ np.asarray(Wq, np.float64).T + np.asarray(bq, np.float64)
    k = Xt @ np.asarray(Wk, np.float64).T + np.asarray(bk, np.float64)
    v = Xt @ np.asarray(Wv, np.float64).T + np.asarray(bv, np.float64)
    s = np.einsum("bsd,btd->bst", q, k) / np.sqrt(np.float64(Xv.shape[-1]))
    s -= s.max(axis=-1, keepdims=True)
    e = np.exp(s)
    attn = e / e.sum(axis=-1, keepdims=True)
    cv = np.einsum("bst,btd->bsd", attn, v)
    ct = np.einsum("bst,bsd->btd", attn, Xv)
    return cv.astype(np.float32), ct.astype(np.float32)


def make_in_maps(vision_repr, text_repr, Wq, bq, Wk, bk, Wv, bv):
    s = 1.0 / np.sqrt(np.float64(DIM))
    m = np.ascontiguousarray(
        (np.asarray(Wq, np.float64).T @ np.asarray(Wk, np.float64) * s)
        .astype(np.float32))
    wvt = np.ascontiguousarray(np.asarray(Wv, np.float32).T)
    vis = np.asarray(vision_repr, np.float32)
    txt = np.asarray(text_repr, np.float32)
    in_maps = []
    for c in range(N_CORES):
        in_maps.append({
            "vision": vis[c * BPC:(c + 1) * BPC],
            "text": txt[c * BPC:(c + 1) * BPC],
            "m_mat": m, "wvt": wvt,
            "ident128": np.eye(P, dtype=np.float32),
        })
    return in_maps


def kernel(vision_repr, text_repr, Wq, bq, Wk, bk, Wv, bv):
    if np.max(np.abs(np.asarray(bq, np.float32))) != 0.0:
        # bq feeds a softmax-variant term the fused-M device path drops;
        # exact host fallback (not hit for this problem: bq is zeros).
        return _reference_numpy(vision_repr, text_repr, Wq, bq, Wk, bk, Wv, bv)

    from concourse.bass_utils import run_bass_kernel_spmd

    nc = _get_nc()
    in_maps = make_in_maps(vision_repr, text_repr, Wq, bq, Wk, bk, Wv, bv)
    res = run_bass_kernel_spmd(nc, in_maps, list(range(N_CORES))).results
    cv = np.concatenate([r_["cross_vision"] for r_ in res], axis=0)
    ct = np.concatenate([r_["cross_text"] for r_ in res], axis=0)
    cv = cv + np.asarray(bv, np.float32)[None, None, :]
    return cv, ct


# revision 7
# speedup vs baseline: 1.1465x; 1.0739x over previous
"""Cross-attention layer (vision<->text) on 8 Trainium2 NeuronCores.

Problem: B=16, Sv=St=1024, D=1024, fp32.
  q = vision @ Wq.T + bq            [B,Sv,D]
  k = text   @ Wk.T + bk            [B,St,D]
  v = text   @ Wv.T + bv            [B,St,D]
  scores = q @ k.T / sqrt(D)        [B,Sv,St]
  attn = softmax(scores, -1)
  cross_vision = attn @ v           [B,Sv,D]
  cross_text   = attn.T @ vision    [B,St,D]

Sharding: pure data-parallel over batch, 2 items per core, no collectives.

Key algebraic restructuring vs the straightforward 6-matmul form:
  scores*sqrt(D) = (Xv Wq^T + 1 bq^T)(Xt Wk^T + 1 bk^T)^T
                 = Xv M' Xt^T + (row-const terms) + 1 (bq^T Wk Xt^T)
  with M' = Wq^T Wk.  Row-constant terms cancel in the row softmax, and
  bq = 0 in this problem (host falls back to an exact numpy path if not),
  so on device scores ~ Xv M Xt^T with M = Wq^T Wk / sqrt(D) precomputed
  on the host.  That replaces {Q proj, K proj, scores} (3x 1024^3 matmuls
  per item) with {T1 = Xv M, scores = T1 Xt^T} (2x), a 1/6 FLOP cut.
  bv is added on the host after gather (attn rows sum to 1, exact).

Precision split: the QK/softmax path runs in float32r (full-rate fp32);
the value/attention-weighting path (E, ET, V, cross products) runs in
bf16 -- attention weights are used linearly so bf16's ~0.4% relative
error lands far inside the 2e-2 gate, and it buys: a bf16 copy of
vision kept SBUF-resident (so cross_text needs NO vision re-stream and
phase H runs DMA-free), half-size E/V tensors, and cheaper E transposes.

Per-core kernel design (per batch item, PE work in parentheses):
  A. prepT: PE-transpose text -> actT[d, t]          (12.3k cyc)
  B. projV: V[t,dv] = actT-stat @ wvt-moving          (65.5k) -- V lands
     directly in the [t, dv] bf16 layout cross_vision needs.
  C. prepV: PE-transpose vision -> actV[d, s]; also   (12.3k)
     cast-copy vision natural -> resident bf16.
  D. T1T[d',s] = M-stat @ actV-moving                 (65.5k) -- M streamed
     from DRAM in column blocks, wvt stays SBUF-resident.
  F/G. per s-tile, software-pipelined at depth 2 so the in-order PE
     never waits on ACT exp or on the DVE/ACT ET-evacuation copies:
     scores[s,t] = T1T-stat @ actT-moving             (65.5k total)
     E = exp(scores) on ACT (bf16 out) + accum row sums; rinv = 1/rowsum
     PE-transpose E row-block -> ET (bf16)            (8.2k total)
     CV[s,dv] = ET-stat @ V-moving, rinv at evac      (65.5k total)
     E *= rinv in place (making attn rows, for CT)
  H. CT[t,d] = E'-stat @ vis_bf16-moving, all-SBUF,   (65.5k)
     accumulated over s in 8 PSUM groups (2 borrowed from the transpose
     pool and evacuated first so the next prep phase isn't blocked).
  All matmuls run at 1.0 cycles/row (moving dim 512).
  Total ~360k PE cycles/item = ~300us/core at 2.4 GHz for 2 items.
"""

import sys

import numpy as np

if "/opt/trn_rl_repo" not in sys.path:
    sys.path.insert(0, "/opt/trn_rl_repo")

import concourse.bass as bass
import concourse.tile as tile
from concourse import bacc
from concourse import mybir

PHASE_MARKS = []  # (phase_name, first_unused_instruction_id) at each boundary

P = 128
B, SEQ, DIM = 16, 1024, 1024
N_CORES = 8
BPC = B // N_CORES  # batch items per core
NT = DIM // P  # 8 tiles of 128 along d
F32 = mybir.dt.float32
F32R = mybir.dt.float32r
BF16 = mybir.dt.bfloat16
AF = mybir.ActivationFunctionType
H = 512  # half of a seq dim / PSUM-bank-sized chunk


class Phases:
    def __init__(self, tc, ident, ident_bf, vis, txt, m_d, wvt_sb, cv_d, ct_d,
                 pools):
        self.tc = tc
        self.nc = tc.nc
        self.ident = ident
        self.ident_bf = ident_bf
        self.vis, self.txt, self.m_d, self.wvt_sb = vis, txt, m_d, wvt_sb
        self.cv_d, self.ct_d = cv_d, ct_d
        (self.p_actT, self.p_ave, self.p_t1, self.p_v, self.p_vbf, self.p_etb,
         self.p_mc, self.p_in, self.p_cvs, self.p_cts, self.p_rp, self.p_rv,
         self.pp_t, self.pp_mm) = pools
        self.actT = {}
        self.v_sb = {}
        self.t1 = {}
        self.e_sb = {}
        self.vis_bf = {}
        self.rinv = {}

    def mark(self, name):
        nid = self.nc._state.next_id()
        PHASE_MARKS.append((name, nid))

    def _prep(self, src_d, pool, tag, b, bf_copy=None):
        """Transpose the full [SEQ, DIM] tensor into actX[d_in, d_out, seq].

        If bf_copy is given, also cast each loaded natural-layout chunk
        into it (a resident [s_in, s_out, d] bf16 copy of the source)."""
        nc = self.nc
        actX = pool.tile([P, NT, SEQ], F32R, name="actX", tag=tag)
        for l in range(NT):
            for hh in range(2):  # two [128, 512] half-row loads, dual queue
                tin = self.p_in.tile([P, H], F32R, name="tin", tag="xin")
                eng = nc.sync if hh == 0 else nc.scalar
                eng.dma_start(
                    out=tin,
                    in_=src_d[b, l * P:(l + 1) * P, hh * H:(hh + 1) * H].bitcast(F32R))
                tp4 = self.pp_t.tile([P, 4, P], F32R, name="tp4", tag="tp4")
                for j in range(4):
                    nc.tensor.matmul(
                        tp4[:, j, :], tin[:, j * P:(j + 1) * P], self.ident,
                        is_transpose=True, start=(j == 0), stop=(j == 3),
                        skip_group_check=True,
                    )
                if hh == 0:
                    nc.vector.tensor_copy(actX[:, 0:4, l * P:(l + 1) * P], tp4)
                    if bf_copy is not None:
                        nc.scalar.copy(bf_copy[:, l, hh * H:(hh + 1) * H], tin)
                else:
                    nc.scalar.copy(actX[:, 4:8, l * P:(l + 1) * P], tp4)
                    if bf_copy is not None:
                        nc.vector.tensor_copy(bf_copy[:, l, hh * H:(hh + 1) * H],
                                              tin)
        return actX

    def prepT(self, b):
        self.mark(f"b{b}_prepT")
        self.actT[b] = self._prep(self.txt, self.p_actT, "actT", b)

    def projV(self, b):
        """V[t, dv] = Xt @ Wv^T, direct [t, dv] layout, bf16 out.

        stat = actT t-block (Xt rows), moving = resident wvt columns."""
        self.mark(f"b{b}_projV")
        nc = self.nc
        actT = self.actT[b]
        v_sb = self.p_v.tile([P, NT, SEQ], BF16, name="v_sb", tag="v")
        for tb in range(NT):
            pss = [self.pp_mm.tile([P, H], F32, name=f"ps_v{i}", tag="mm")
                   for i in range(2)]
            for do in range(NT):
                for hh in range(2):
                    nc.tensor.matmul(pss[hh], actT[:, do, tb * P:(tb + 1) * P],
                                     self.wvt_sb[:, do, hh * H:(hh + 1) * H],
                                     start=(do == 0), stop=(do == NT - 1))
            nc.vector.tensor_copy(v_sb[:, tb, 0:H], pss[0])
            nc.scalar.copy(v_sb[:, tb, H:2 * H], pss[1])
        self.v_sb[b] = v_sb

    def prepV(self, b):
        self.mark(f"b{b}_prepV")
        vis_bf = self.p_vbf.tile([P, NT, SEQ], BF16, name="vis_bf", tag="vbf")
        self.vis_bf[b] = vis_bf
        self.actV = self._prep(self.vis, self.p_ave, "ave", b, bf_copy=vis_bf)

    def t1t(self, b):
        """T1T[d', s] = (Xv M)^T = M-colblock-stat @ actV, M streamed."""
        self.mark(f"b{b}_T1")
        nc = self.nc
        t1 = self.p_t1.tile([P, NT, SEQ], F32R, name="t1", tag="t1")
        for eo in range(NT):
            mc = self.p_mc.tile([P, NT, P], F32R, name="mc", tag="mc")
            nc.scalar.dma_start(
                out=mc,
                in_=self.m_d[:, eo * P:(eo + 1) * P]
                    .rearrange("(do di) e -> di do e", di=P),
            )
            pss = [self.pp_mm.tile([P, H], F32, name=f"ps_t{i}", tag="mm")
                   for i in range(2)]
            for do in range(NT):
                for hh in range(2):
                    nc.tensor.matmul(pss[hh], mc[:, do, :],
                                     self.actV[:, do, hh * H:(hh + 1) * H],
                                     start=(do == 0), stop=(do == NT - 1))
            nc.vector.tensor_copy(t1[:, eo, 0:H], pss[0])
            nc.scalar.copy(t1[:, eo, H:2 * H], pss[1])
        self.t1[b] = t1

    def fg(self, b):
        """scores -> exp/rowsum -> ET -> cross_vision, depth-2 pipelined.

        PE order per steady-state iteration: scores(so+2) matmuls,
        E-transposes(so+1), CV(so) matmuls.  exp(so+2) runs on ACT during
        the next iteration's scores; the ET copies of so+1 complete during
        scores(so+2)/CV(so), so CV(so+1) never waits on them."""
        self.mark(f"b{b}_F")
        nc = self.nc
        t1, actT, v_sb = self.t1[b], self.actT[b], self.v_sb[b]
        e_sb = self.p_ave.tile([P, NT, SEQ], BF16, name="e_sb", tag="ave")
        rinv = self.p_rv.tile([P, NT], F32, name="rinv", tag="rinv")
        self.e_sb[b] = e_sb
        rps = {}
        etbs = {}

        def scores_stile(so):
            rp = self.p_rp.tile([P, 2], F32, name="rp", tag="rp")
            pss = [self.pp_mm.tile([P, H], F32, name=f"ps_s{i}", tag="mm")
                   for i in range(2)]
            for do in range(NT):
                for tc_ in range(2):
                    nc.tensor.matmul(pss[tc_], t1[:, do, so * P:(so + 1) * P],
                                     actT[:, do, tc_ * H:(tc_ + 1) * H],
                                     start=(do == 0), stop=(do == NT - 1))
            for tc_ in range(2):
                nc.scalar.activation(out=e_sb[:, so, tc_ * H:(tc_ + 1) * H],
                                     in_=pss[tc_], func=AF.Exp,
                                     accum_out=rp[:, tc_:tc_ + 1])
            rps[so] = rp

        def etp(so):
            rp = rps.pop(so)
            rsum = self.p_rp.tile([P, 1], F32, name="rsum", tag="rsum")
            nc.vector.tensor_add(rsum, rp[:, 0:1], rp[:, 1:2])
            nc.vector.reciprocal(rinv[:, so:so + 1], rsum)
            # transpose the *unnormalized* E row-block; copies split across
            # DVE and ACT so CV's first stat is ready fast
            etb = self.p_etb.tile([P, NT, P], BF16, name="etb", tag="etb")
            for tg in range(2):
                tp4 = self.pp_t.tile([P, 4, P], BF16, name="tp4e", tag="tp4")
                for j in range(4):
                    tt = tg * 4 + j
                    nc.tensor.matmul(tp4[:, j, :], e_sb[:, so, tt * P:(tt + 1) * P],
                                     self.ident_bf, is_transpose=True,
                                     start=(j == 0), stop=(j == 3),
                                     skip_group_check=True)
                nc.vector.tensor_copy(etb[:, tg * 4:tg * 4 + 2, :], tp4[:, 0:2, :])
                nc.scalar.copy(etb[:, tg * 4 + 2:tg * 4 + 4, :], tp4[:, 2:4, :])
            etbs[so] = etb

        def cv(so):
            etb = etbs.pop(so)
            pcv = [self.pp_mm.tile([P, H], F32, name=f"ps_cv{i}", tag="mm")
                   for i in range(2)]
            for tt in range(NT):
                for dc in range(2):
                    nc.tensor.matmul(pcv[dc], etb[:, tt, :],
                                     v_sb[:, tt, dc * H:(dc + 1) * H],
                                     start=(tt == 0), stop=(tt == NT - 1))
            for dc in range(2):
                cvs = self.p_cvs.tile([P, H], F32, name="cvs", tag="cvs")
                nc.scalar.mul(cvs, pcv[dc], mul=rinv[:, so:so + 1])
                nc.gpsimd.dma_start(
                    out=self.cv_d[b, so * P:(so + 1) * P, dc * H:(dc + 1) * H],
                    in_=cvs)
            # normalize this E row-block in place (for cross_text later)
            nc.vector.tensor_scalar_mul(e_sb[:, so, :], e_sb[:, so, :],
                                        scalar1=rinv[:, so:so + 1])

        scores_stile(0)
        scores_stile(1)
        etp(0)
        for so in range(NT):
            if so + 2 < NT:
                scores_stile(so + 2)
            if so + 1 < NT:
                etp(so + 1)
            cv(so)
        self.rinv[b] = rinv

    def h(self, b, last):
        """cross_text = E'.T @ vis_bf16 (E' already rinv-scaled), all-SBUF.

        Per d-half: 8 concurrent PSUM accumulation groups (2 borrowed from
        the transpose pool, placed FIRST so their evac copies free them
        immediately for the next prep phase).  The final d-half's stores
        split across the sync+gpsimd queues to shorten the kernel tail."""
        self.mark(f"b{b}_H")
        nc = self.nc
        e_sb = self.e_sb[b]
        vis_bf = self.vis_bf[b]
        for dc in range(2):
            pss = [self.pp_t.tile([P, H], F32, name=f"ps_ct{i}", tag="tp4")
                   for i in range(2)]
            pss += [self.pp_mm.tile([P, H], F32, name=f"ps_ct{i + 2}", tag="mm")
                    for i in range(6)]
            for so in range(NT):
                for tt in range(NT):
                    nc.tensor.matmul(pss[tt], e_sb[:, so, tt * P:(tt + 1) * P],
                                     vis_bf[:, so, dc * H:(dc + 1) * H],
                                     start=(so == 0), stop=(so == NT - 1))
            final = last and dc == 1
            for tt in range(NT):
                cts = self.p_cts.tile([P, H], F32, name="cts", tag="cts")
                if tt % 2 == 0:
                    nc.vector.tensor_copy(cts, pss[tt])
                else:
                    nc.scalar.copy(cts, pss[tt])
                eng = nc.sync if (final and tt % 2 == 0) else nc.gpsimd
                eng.dma_start(
                    out=self.ct_d[b, tt * P:(tt + 1) * P, dc * H:(dc + 1) * H],
                    in_=cts)
        self.mark(f"b{b}_end")


def build_nc():
    nc = bacc.Bacc("TRN2", target_bir_lowering=False, debug=False, num_devices=N_CORES)
    vis = nc.dram_tensor("vision", [BPC, SEQ, DIM], F32, kind="ExternalInput").ap()
    txt = nc.dram_tensor("text", [BPC, SEQ, DIM], F32, kind="ExternalInput").ap()
    m_d = nc.dram_tensor("m_mat", [DIM, DIM], F32R, kind="ExternalInput").ap()
    wvt_d = nc.dram_tensor("wvt", [DIM, DIM], F32R, kind="ExternalInput").ap()
    id_d = nc.dram_tensor("ident128", [P, P], F32R, kind="ExternalInput").ap()
    cv_d = nc.dram_tensor("cross_vision", [BPC, SEQ, DIM], F32, kind="ExternalOutput").ap()
    ct_d = nc.dram_tensor("cross_text", [BPC, SEQ, DIM], F32, kind="ExternalOutput").ap()

    with tile.TileContext(nc) as tc:
        import contextlib
        with contextlib.ExitStack() as ctx:
            def sp(name, bufs):
                return ctx.enter_context(tc.tile_pool(name=name, bufs=bufs))

            p_actT = sp("actT", 1)   # 32KB/part: Xt^T (f32r)
            p_ave = sp("ave", 1)     # 32KB/part: actV (f32r) then e_sb (bf16)
            p_t1 = sp("t1", 1)       # 32KB/part: T1^T (f32r)
            p_v = sp("v", 1)         # 16KB/part: V[t, dv] (bf16)
            p_vbf = sp("vbf", 1)     # 16KB/part: resident bf16 vision natural
            p_w = sp("w", 1)         # 32KB/part: resident Wv^T (f32r)
            p_etb = sp("etb", 2)     # 4KB/part: ET blocks (bf16), depth-2
            p_mc = sp("mc", 2)       # 8KB/part: M column-block staging
            p_in = sp("xin", 8)      # 16KB/part: prep load staging
            p_cvs = sp("cvs", 2)     # 4KB/part
            p_cts = sp("cts", 4)     # 8KB/part
            p_rp = sp("rp", 4)
            p_rv = sp("rv", 2)
            p_sm = sp("sm", 1)
            pp_t = ctx.enter_context(
                tc.tile_pool(name="pp_t", bufs=2, space=bass.MemorySpace.PSUM))
            pp_mm = ctx.enter_context(
                tc.tile_pool(name="pp_mm", bufs=6, space=bass.MemorySpace.PSUM))

            ident = p_sm.tile([P, P], F32R, name="ident")
            nc.sync.dma_start(out=ident, in_=id_d)
            ident_bf = p_sm.tile([P, P], BF16, name="ident_bf")
            nc.vector.tensor_copy(ident_bf, ident)
            # resident Wv^T [di, do, e]: 8 plain contiguous 512KB loads
            wvt_sb = p_w.tile([P, NT, SEQ], F32R, name="wvt_sb", tag="wvt")
            for do in range(NT):
                nc.gpsimd.dma_start(out=wvt_sb[:, do, :],
                                    in_=wvt_d[do * P:(do + 1) * P, :])

            pools = (p_actT, p_ave, p_t1, p_v, p_vbf, p_etb, p_mc, p_in,
                     p_cvs, p_cts, p_rp, p_rv, pp_t, pp_mm)
            ph = Phases(tc, ident, ident_bf, vis, txt, m_d, wvt_sb, cv_d, ct_d,
                        pools)
            ph.prepT(0)
            for b in range(BPC):
                ph.projV(b)
                ph.prepV(b)
                ph.t1t(b)
                ph.fg(b)
                ph.h(b, last=(b + 1 == BPC))
                if b + 1 < BPC:
                    ph.prepT(b + 1)
    nc.compile()
    return nc


_NC_CACHE = None


def _get_nc():
    global _NC_CACHE
    if _NC_CACHE is None:
        _NC_CACHE = build_nc()
    return _NC_CACHE


def _reference_numpy(vision_repr, text_repr, Wq, bq, Wk, bk, Wv, bv):
    """Exact fallback (never hit for this problem's inputs: bq == 0)."""
    Xv = np.asarray(vision_repr, np.float64)
    Xt = np.asarray(text_repr, np.float64)
    q = Xv @ np.asarray(Wq, np.float64).T + np.asarray(bq, np.float64)
    k = Xt @ np.asarray(Wk, np.float64).T + np.asarray(bk, np.float64)
    v = Xt @ np.asarray(Wv, np.float64).T + np.asarray(bv, np.float64)
    s = np.einsum("bsd,btd->bst", q, k) / np.sqrt(np.float64(Xv.shape[-1]))
    s -= s.max(axis=-1, keepdims=True)
    e = np.exp(s)
    attn = e / e.sum(axis=-1, keepdims=True)
    cv = np.einsum("bst,btd->bsd", attn, v)
    ct = np.einsum("bst,bsd->btd", attn, Xv)
    return cv.astype(np.float32), ct.astype(np.float32)


def make_in_maps(vision_repr, text_repr, Wq, bq, Wk, bk, Wv, bv):
    s = 1.0 / np.sqrt(np.float64(DIM))
    m = np.ascontiguousarray(
        (np.asarray(Wq, np.float64).T @ np.asarray(Wk, np.float64) * s)
        .astype(np.float32))
    wvt = np.ascontiguousarray(np.asarray(Wv, np.float32).T)
    vis = np.asarray(vision_repr, np.float32)
    txt = np.asarray(text_repr, np.float32)
    in_maps = []
    for c in range(N_CORES):
        in_maps.append({
            "vision": vis[c * BPC:(c + 1) * BPC],
            "text": txt[c * BPC:(c + 1) * BPC],
            "m_mat": m, "wvt": wvt,
            "ident128": np.eye(P, dtype=np.float32),
        })
    return in_maps


def kernel(vision_repr, text_repr, Wq, bq, Wk, bk, Wv, bv):
    if np.max(np.abs(np.asarray(bq, np.float32))) != 0.0:
        # bq feeds a softmax-variant term the fused-M device path drops;
        # exact host fallback (not hit for this problem: bq is zeros).
        return _reference_numpy(vision_repr, text_repr, Wq, bq, Wk, bk, Wv, bv)

    from concourse.bass_utils import run_bass_kernel_spmd

    nc = _get_nc()
    in_maps = make_in_maps(vision_repr, text_repr, Wq, bq, Wk, bk, Wv, bv)
    res = run_bass_kernel_spmd(nc, in_maps, list(range(N_CORES))).results
    cv = np.concatenate([r_["cross_vision"] for r_ in res], axis=0)
    ct = np.concatenate([r_["cross_text"] for r_ in res], axis=0)
    cv = cv + np.asarray(bv, np.float32)[None, None, :]
    return cv, ct
